# revision 4
# baseline (speedup 1.0000x reference)
"""Trainium2 Bass kernel for nn_PolyAttn (B=4, N=2048, D=H=1024).

Mathematical structure exploited: the reference computes attention weights
a = (alpha*q@k^T + 1)^4 followed by a = a / |a|.  Since s^4 >= 0, the
normalized score matrix is exactly the all-ones matrix (independent of
alpha), so

    o[b, n, :] = (sum_m x[b, m, :]) @ W_v @ w_o        for every n,

where W_v = w_qkv[:, 2H:3H].  The device kernels compute exactly this:

  Launch 1 (all 8 cores): each core reduces a distinct 1024-row slice of
     x (flattened to [8192, 1024]) down to a row-sum px [1, 1024]:
     DVE add-tree folds 8 [128, 1024] tiles into one, then a PE matmul
     against a ones-vector reduces the 128 partitions.
     Host sums pairs of partials (the cross-core reduce) -> xs [4, 1024].
  Launch 2 (all 8 cores): weights are sharded over the hidden dim; core i
     owns W_v[:, 128i:128(i+1)] and w_o[128i:128(i+1), :] and computes its
     rank-128 contribution r_i = (xs @ Wv_i) @ wo_i  [4, 1024].
     Host sums the 8 partials -> r, and broadcasts r over the sequence dim
     to the full [4, 2048, 1024] output.
"""

import numpy as np

import concourse.bacc as bacc
import concourse.mybir as mybir
import concourse.tile as tile
from concourse.bass_utils import run_bass_kernel_spmd

NCORES = 8
B, N, D, H = 4, 2048, 1024, 1024
F32 = mybir.dt.float32
CHUNK = H // NCORES  # 128 hidden channels per core in launch 2

_BUILT = {}


def _build_l1():
    """Row-reduce x-slice [1024, 1024] into px [1, 1024]."""
    nc = bacc.Bacc("TRN2", target_bir_lowering=False, debug=False,
                   num_devices=NCORES)
    xs_ = nc.dram_tensor("xslice", [1024, 1024], F32, kind="ExternalInput")
    ones = nc.dram_tensor("ones", [128, 1], F32, kind="ExternalInput")
    px = nc.dram_tensor("px", [1, 1024], F32, kind="ExternalOutput")

    with tile.TileContext(nc) as tc:
        with (
            tc.tile_pool(name="sbuf", bufs=8) as pool,
            tc.tile_pool(name="mid", bufs=7) as mid,
            tc.tile_pool(name="cst", bufs=1) as cst,
            tc.tile_pool(name="psum", bufs=1, space="PSUM") as psum,
        ):
            ones_sb = cst.tile([128, 1], F32)
            nc.sync.dma_start(ones_sb[:], ones[:])
            xts = []
            for j in range(8):
                xt = pool.tile([128, 1024], F32)
                nc.sync.dma_start(xt[:], xs_[128 * j : 128 * (j + 1), :])
                xts.append(xt)
            # halves pipelined so PE can start on cols [:512] while DVE
            # finishes cols [512:]
            acc = cst.tile([128, 1024], F32)
            pa = psum.tile([1, 512], F32)
            pb = psum.tile([1, 512], F32)
            po = cst.tile([1, 1024], F32)
            for h, (lo, hi, ps) in enumerate(((0, 512, pa), (512, 1024, pb))):
                s01 = mid.tile([128, 512], F32)
                s23 = mid.tile([128, 512], F32)
                s45 = mid.tile([128, 512], F32)
                s67 = mid.tile([128, 512], F32)
                nc.vector.tensor_add(s01[:], xts[0][:, lo:hi], xts[1][:, lo:hi])
                nc.vector.tensor_add(s23[:], xts[2][:, lo:hi], xts[3][:, lo:hi])
                nc.vector.tensor_add(s45[:], xts[4][:, lo:hi], xts[5][:, lo:hi])
                nc.vector.tensor_add(s67[:], xts[6][:, lo:hi], xts[7][:, lo:hi])
                s03 = mid.tile([128, 512], F32)
                s47 = mid.tile([128, 512], F32)
                nc.vector.tensor_add(s03[:], s01[:], s23[:])
                nc.vector.tensor_add(s47[:], s45[:], s67[:])
                nc.vector.tensor_add(acc[:, lo:hi], s03[:], s47[:])
                nc.tensor.matmul(ps[:], ones_sb[:], acc[:, lo:hi])
                nc.vector.tensor_copy(po[:, lo:hi], ps[:])
            nc.sync.dma_start(px[:], po[:])
    nc.compile()
    return nc


def _build_l2():
    """r_part [4, 1024] = (xs @ Wv_chunk) @ wo_chunk for this core's chunk.

    xsT: xs pre-transposed on host, [1024, 4] viewed as 8 K-tiles [128, 4].
    wv:  W_v[:, chunk] as [1024, 128], viewed as 8 K-tiles [128, 128].
    wo:  w_o[chunk, :] as [128, 1024].

    t [4, 128] accumulates xsT_a.T @ wv_a (stationary operand is the tiny
    xsT tile, so no expensive f32 weight loads); t is then PE-transposed
    and used as the stationary operand against wo.
    """
    nc = bacc.Bacc("TRN2", target_bir_lowering=False, debug=False,
                   num_devices=NCORES)
    xsT = nc.dram_tensor("xsT", [1024, 4], F32, kind="ExternalInput")
    wv = nc.dram_tensor("wv", [1024, 128], F32, kind="ExternalInput")
    wo = nc.dram_tensor("wo", [128, 1024], F32, kind="ExternalInput")
    id4 = nc.dram_tensor("id4", [4, 4], F32, kind="ExternalInput")
    rp = nc.dram_tensor("rpart", [4, 1024], F32, kind="ExternalOutput")

    with tile.TileContext(nc) as tc:
        with (
            tc.tile_pool(name="sbuf", bufs=1) as pool,
            tc.tile_pool(name="psum", bufs=1, space="PSUM") as psum,
        ):
            id4_sb = pool.tile([4, 4], F32)
            nc.sync.dma_start(id4_sb[:], id4[:])
            xsT_sb = pool.tile([128, 8, 4], F32)
            nc.sync.dma_start(
                xsT_sb[:], xsT.ap().rearrange("(a p) c -> p a c", p=128)
            )
            wv_sb = pool.tile([128, 8, 128], F32)
            nc.sync.dma_start(
                wv_sb[:], wv.ap().rearrange("(a p) c -> p a c", p=128)
            )
            wo_sb = pool.tile([128, 1024], F32)
            nc.sync.dma_start(wo_sb[:], wo[:])

            # t [4, 128] = sum_a xsT_a.T @ wv_a  (= xs @ Wv_chunk)
            pt = psum.tile([4, 128], F32)
            for a in range(8):
                nc.tensor.matmul(pt[:], xsT_sb[:, a, :], wv_sb[:, a, :],
                                 start=(a == 0), stop=(a == 7))
            t_sb = pool.tile([4, 128], F32)
            nc.vector.tensor_copy(t_sb[:], pt[:])

            # tT [128, 4] via PE transpose
            ptT = psum.tile([128, 4], F32)
            nc.tensor.transpose(ptT[:], t_sb[:], id4_sb[:])
            tT_sb = pool.tile([128, 4], F32)
            nc.vector.tensor_copy(tT_sb[:], ptT[:])

            # r_part [4, 1024] = tT.T @ wo_chunk
            pra = psum.tile([4, 512], F32)
            prb = psum.tile([4, 512], F32)
            nc.tensor.matmul(pra[:], tT_sb[:], wo_sb[:, :512])
            nc.tensor.matmul(prb[:], tT_sb[:], wo_sb[:, 512:])
            ro = pool.tile([4, 1024], F32)
            nc.vector.tensor_copy(ro[:, :512], pra[:])
            nc.vector.tensor_copy(ro[:, 512:], prb[:])
            nc.sync.dma_start(rp[:], ro[:])
    nc.compile()
    return nc


def _get(name, builder):
    if name not in _BUILT:
        _BUILT[name] = builder()
    return _BUILT[name]


def kernel(x, w_qkv, w_o, alpha):
    x = np.ascontiguousarray(np.asarray(x, dtype=np.float32))
    w_qkv = np.asarray(w_qkv, dtype=np.float32)
    w_o = np.ascontiguousarray(np.asarray(w_o, dtype=np.float32))
    core_ids = list(range(NCORES))

    # ---- Launch 1: row-reduce x across all 8 cores -----------------------
    nc1 = _get("l1", _build_l1)
    xflat = x.reshape(B * N, D)  # rows [1024*i : 1024*(i+1)) belong to batch i//2
    ones = np.ones((128, 1), dtype=np.float32)
    in_maps1 = [
        {"xslice": xflat[1024 * i : 1024 * (i + 1)], "ones": ones}
        for i in range(NCORES)
    ]
    res1 = run_bass_kernel_spmd(nc1, in_maps1, core_ids)
    pxs = [r["px"][0] for r in res1.results]
    # core 2b and 2b+1 each reduced one half of batch b
    xs = np.stack([pxs[2 * b] + pxs[2 * b + 1] for b in range(B)])  # [4, 1024]

    # ---- Launch 2: (xs @ Wv_chunk) @ wo_chunk, hidden dim sharded --------
    nc2 = _get("l2", _build_l2)
    xsT = np.ascontiguousarray(xs.T)  # [1024, 4]
    id4 = np.eye(4, dtype=np.float32)
    in_maps2 = []
    for i in range(NCORES):
        c0, c1 = CHUNK * i, CHUNK * (i + 1)
        in_maps2.append({
            "xsT": xsT,
            "wv": np.ascontiguousarray(w_qkv[:, 2 * H + c0 : 2 * H + c1]),
            "wo": np.ascontiguousarray(w_o[c0:c1, :]),
            "id4": id4,
        })
    res2 = run_bass_kernel_spmd(nc2, in_maps2, core_ids)
    r = np.sum([res["rpart"] for res in res2.results], axis=0)  # [4, 1024]

    # ---- Unshard: the score-normalized attention is all-ones, so every
    # sequence position of batch b carries the same row r[b].
    out = np.broadcast_to(r[:, None, :], (B, N, D))
    return np.ascontiguousarray(out)
